# revision 19
# baseline (speedup 1.0000x reference)
"""Trainium2 Bass kernel for 3-layer ConvLSTM2D stack + BN/ReLU + Conv3D+sigmoid.

Model (per reference):
  for l in 0..2:  h = bn_relu(conv_lstm(h, k_l, rk_l, b_l), g_l, be_l)
  y = sigmoid(conv3d(h, w3) + b3)

Shapes: x [B=8, T=12, 64, 64, 1], F=64 filters per layer, 3x3 kernels, SAME.

Sharding: data-parallel over B across 8 NeuronCores (1 image/core), weights
replicated.

Layout: channels on SBUF partitions, pixels on the free axis in zero-padded
66x66 frames (flat 4356, stride 4360). A 3x3 conv = 9 shifted-tap matmuls
accumulated in PSUM (K=chans, M=out-chans, N=pixels), taps K-packed 2-per-
matmul via shifted replicas on the other 64 partitions of the moving tile:
  tile A = (src, src shifted +1); tile B = (src, src shifted +66)
All matmul operands are bf16 (enables fast weight load); PSUM accumulates
fp32; the c state stays fp32 in SBUF.

Gates: z lives in PSUM as psA=[i;f], psB=[g;o]. c/h reside on partitions
64:128 (same as f,o); the DVE computes i*tanh(g) with a cross-partition
write directly onto 64:128, so no relocation matmul is needed. BN+ReLU is
one ScalarE op (scale=g/sqrt(1+eps), bias=be).

Pipeline: per timestep the 8 pixel chunks are processed in pairs with the
gate math lagging one pair behind the matmuls; h-replica and stage-output
DMAs are issued per quarter-frame piece as soon as their source chunks are
done, so timestep t+1's matmuls start while t's gate tail is still in
flight. Frame loads run on the gpsimd queue, replica/stage pieces on sync,
keeping the scalar queue free for activations.
"""
import sys
import os

_REPO = '/opt/trn_rl_repo'
if _REPO not in sys.path:
    sys.path.insert(0, _REPO)

import numpy as np  # noqa: E402
import ml_dtypes  # noqa: E402

T, H, W, F = 12, 64, 64, 64
B = 8
PW = H + 2                 # padded width = 66
FRAME = PW * PW            # 4356
FSTRIDE = FRAME + 4        # frame stride per channel = 4360
NCHUNK = 8
CROWS = H // NCHUNK        # 8 rows per chunk
CHUNK = CROWS * W          # 512 pixels per chunk
INTER = PW + 1             # interior base offset = 67
XLEAD = PW + 1
XLEN = XLEAD + T * FRAME + 160

PAIRS_A = [((0, 0), (0, 1)), ((1, 0), (1, 1)), ((2, 0), (2, 1))]
PAIR_B = ((0, 2), (1, 2))
SINGLE_A = (2, 2)
REC_SINGLES = [(0, 2), (1, 2), (2, 2)]


def _off(t):
    return (t[0] - 1) * PW + (t[1] - 1)


A_PAIR_BASE = [INTER + _off(p[0]) for p in PAIRS_A]          # 0, 66, 132
B_PAIR_BASE = INTER + _off(PAIR_B[0])                        # 2
A_SINGLE_BASE = INTER + _off(SINGLE_A)                       # 134
REC_SINGLE_BASE = [INTER + _off(t) for t in REC_SINGLES]     # 2, 68, 134

BF16 = ml_dtypes.bfloat16


def pack_weights(ks, rks):
    """Pack conv weights.

    Returns (wt_inp[bf16], wt_rec[bf16], groups); groups[layer] =
    {'inp': [(i0, i1, kp, in_tile, base)], 'rec': [...]}; in_tile 0=A 1=B;
    kp: 128 pair, 64 single (lower), -64 single (upper, h_cur data half).
    """
    ti, tr = [], []
    groups = []

    def addt(lst, w):
        lst.append(w)
        return len(lst) - 1

    def pair(k, ta, tb, m, swap):
        w = np.zeros((128, 128), np.float32)
        lo, hi = (tb, ta) if swap else (ta, tb)
        w[0:64, :] = k[lo[0], lo[1], :, m * 128:(m + 1) * 128]
        w[64:128, :] = k[hi[0], hi[1], :, m * 128:(m + 1) * 128]
        return w

    def single(k, t, m, upper):
        w = np.zeros((128, 128), np.float32)
        r0 = 64 if upper else 0
        w[r0:r0 + 64, :] = k[t[0], t[1], :, m * 128:(m + 1) * 128]
        return w

    for layer in range(3):
        k, rk = ks[layer], rks[layer]
        g = {'inp': [], 'rec': []}
        if layer == 0:
            k9 = k.reshape(9, 4 * F)
            idx = []
            for m in range(2):
                w = np.zeros((128, 128), np.float32)
                w[0:9, :] = k9[:, m * 128:(m + 1) * 128]
                idx.append(addt(ti, w))
            g['inp'].append((idx[0], idx[1], 9, 0, 0))
        else:
            for i, (ta, tb) in enumerate(PAIRS_A):
                g['inp'].append((addt(ti, pair(k, ta, tb, 0, False)),
                                 addt(ti, pair(k, ta, tb, 1, False)),
                                 128, 0, A_PAIR_BASE[i]))
            g['inp'].append((addt(ti, pair(k, PAIR_B[0], PAIR_B[1], 0,
                                           False)),
                             addt(ti, pair(k, PAIR_B[0], PAIR_B[1], 1,
                                           False)),
                             128, 1, B_PAIR_BASE))
            g['inp'].append((addt(ti, single(k, SINGLE_A, 0, False)),
                             addt(ti, single(k, SINGLE_A, 1, False)),
                             64, 0, A_SINGLE_BASE))
        for i, (ta, tb) in enumerate(PAIRS_A):
            g['rec'].append((addt(tr, pair(rk, ta, tb, 0, True)),
                             addt(tr, pair(rk, ta, tb, 1, True)),
                             128, 0, A_PAIR_BASE[i]))
        # B-structure pair and single read the hB tile (h data lower,
        # h shifted +66 upper) — same layout as the input-path tiles
        g['rec'].append((addt(tr, pair(rk, PAIR_B[0], PAIR_B[1], 0, False)),
                         addt(tr, pair(rk, PAIR_B[0], PAIR_B[1], 1, False)),
                         128, 2, B_PAIR_BASE))
        g['rec'].append((addt(tr, single(rk, SINGLE_A, 0, False)),
                         addt(tr, single(rk, SINGLE_A, 1, False)),
                         64, 2, A_SINGLE_BASE))
        groups.append(g)

    return (np.stack(ti).astype(BF16), np.stack(tr).astype(BF16),
            groups)


N_WTI = 22
N_WTR = 30  # 10 * 3


def pack_w3(w3):
    """conv3d weights -> bf16 [5,128,4] tiles (3 cols used) + table."""
    w3 = w3[:, :, :, :, 0]  # [3(dt), 3, 3, 64]
    tiles = np.zeros((5, 128, 4), np.float32)
    table = []
    for i, (ta, tb) in enumerate(PAIRS_A):
        for m in range(3):
            tiles[i, 0:64, m] = w3[m, ta[0], ta[1], :]
            tiles[i, 64:128, m] = w3[m, tb[0], tb[1], :]
        table.append((i, 128, 0, A_PAIR_BASE[i]))
    for m in range(3):
        tiles[3, 0:64, m] = w3[m, PAIR_B[0][0], PAIR_B[0][1], :]
        tiles[3, 64:128, m] = w3[m, PAIR_B[1][0], PAIR_B[1][1], :]
    table.append((3, 128, 1, B_PAIR_BASE))
    for m in range(3):
        tiles[4, 0:64, m] = w3[m, SINGLE_A[0], SINGLE_A[1], :]
    table.append((4, 64, 0, A_SINGLE_BASE))
    return tiles.astype(BF16), table


def build_nc(TT=T):
    import concourse.bass as bass
    import concourse.mybir as mybir
    import concourse.tile as tile
    from concourse import bacc

    F32, F32R, BF = mybir.dt.float32, mybir.dt.float32r, mybir.dt.bfloat16
    AF = mybir.ActivationFunctionType

    nc = bacc.Bacc("TRN2", target_bir_lowering=False, debug=False,
                   num_devices=8)

    d_x = nc.dram_tensor("x_im", [1, XLEN], BF, kind="ExternalInput")
    d_wti = nc.dram_tensor("wti", [N_WTI, 128, 128], BF, kind="ExternalInput")
    d_wtr = nc.dram_tensor("wtr", [N_WTR, 128, 128], BF,
                           kind="ExternalInput")
    d_w3 = nc.dram_tensor("w3t", [5, 128, 4], BF, kind="ExternalInput")
    d_bd = nc.dram_tensor("bd", [3 * TT, TT], BF, kind="ExternalInput")
    d_b = nc.dram_tensor("b_all", [3, 256], F32, kind="ExternalInput")
    d_gb = nc.dram_tensor("gb_all", [3, 64], F32, kind="ExternalInput")
    d_be = nc.dram_tensor("be_all", [3, 64], F32, kind="ExternalInput")
    d_b3 = nc.dram_tensor("b3b", [TT, 1], F32, kind="ExternalInput")
    d_y = nc.dram_tensor("y", [TT, H * W], F32, kind="ExternalOutput")
    DBG = bool(os.environ.get("KDBG"))
    d_dbg = [nc.dram_tensor(f"dbg{l}", [TT, 64, FRAME], BF,
                            kind="ExternalOutput") if DBG else None
             for l in range(3)]

    _, _, wgroups = pack_weights(
        [np.zeros((3, 3, 1, 256), np.float32)] +
        [np.zeros((3, 3, 64, 256), np.float32)] * 2,
        [np.zeros((3, 3, 64, 256), np.float32)] * 3)
    _, w3table = pack_w3(np.zeros((3, 3, 3, 64, 1), np.float32))

    def sub_ap(tile_obj, p0, np_, free_off, free_dims):
        base = tile_obj[:]
        ps = base.ap[0][0]
        return bass.AP(base.tensor, base.offset + p0 * ps + free_off,
                       [[ps, np_]] + [list(d) for d in free_dims])

    def conv_rhs(tile_obj, kparts, base, chunk):
        return sub_ap(tile_obj, 0, kparts, base + chunk * CROWS * PW,
                      [[PW, CROWS], [1, W]])

    def interior_ap(tile_obj, chunk, nchunks=1, p0=64):
        return sub_ap(tile_obj, p0, 64, INTER + chunk * CROWS * PW,
                      [[PW, nchunks * CROWS], [1, W]])

    with tile.TileContext(nc) as tc:
        with tc.tile_pool(name="small", bufs=1) as small, \
             tc.tile_pool(name="gates", bufs=3) as gates, \
             tc.tile_pool(name="c3s", bufs=2) as c3s, \
             tc.tile_pool(name="fr", bufs=2) as frp, \
             tc.tile_pool(name="dram", bufs=1, space="DRAM") as dpool:

            b_if, b_go, gb_t, be_t = [], [], [], []
            for l in range(3):
                tt1 = small.tile([128, 1], F32, tag=f"bif{l}")
                nc.sync.dma_start(
                    tt1[:], d_b[:][l, 0:128].rearrange("(a o) -> a o", o=1))
                b_if.append(tt1)
                tt2 = small.tile([128, 1], F32, tag=f"bgo{l}")
                nc.sync.dma_start(
                    tt2[:], d_b[:][l, 128:256].rearrange("(a o) -> a o", o=1))
                b_go.append(tt2)
                tt4 = small.tile([128, 1], F32, tag=f"gb{l}")
                nc.sync.dma_start(
                    tt4[64:128, :],
                    d_gb[:][l, :].rearrange("(a o) -> a o", o=1))
                gb_t.append(tt4)
                tt5 = small.tile([128, 1], F32, tag=f"be{l}")
                nc.sync.dma_start(
                    tt5[64:128, :],
                    d_be[:][l, :].rearrange("(a o) -> a o", o=1))
                be_t.append(tt5)
            b3t = small.tile([TT, 1], F32, tag="b3")
            nc.sync.dma_start(b3t[:], d_b3[:])

            scratch = [dpool.tile([TT, 64, FSTRIDE], BF, tag=f"scr{i}",
                                  name=f"scr{i}") for i in range(2)]

            wti_t, wtr_t = {}, {}
            for i in range(N_WTI):
                wt_ = small.tile([128, 128], BF, tag=f"wti{i}",
                                 name=f"wti{i}")
                nc.sync.dma_start(wt_[:], d_wti[:][i, :, :])
                wti_t[i] = wt_
            for i in range(N_WTR):
                wt_ = small.tile([128, 128], BF, tag=f"wtr{i}",
                                 name=f"wtr{i}")
                nc.gpsimd.dma_start(wt_[:], d_wtr[:][i, :, :])
                wtr_t[i] = wt_
            w3tiles = []
            for i in range(5):
                w3i = small.tile([128, 4], BF, tag=f"w3_{i}")
                nc.sync.dma_start(w3i[:], d_w3[:][i, :, :])
                w3tiles.append(w3i)
            bdt = small.tile([3 * TT, TT], BF, tag="bd")
            nc.sync.dma_start(bdt[:], d_bd[:])

            def frame_dma_A(dst, scr, t, eng):
                src = scr[:][t, :, :].rearrange("a b -> (a b)")
                ap = bass.AP(src.tensor, src.offset,
                             [[1, 2], [FSTRIDE, 64], [1, FRAME]])
                eng.dma_start(dst[:, 0:FRAME], ap)

            def frame_dma_B(dst, scr, t, eng):
                src = scr[:][t, :, :].rearrange("a b -> (a b)")
                ap = bass.AP(src.tensor, src.offset,
                             [[PW, 2], [FSTRIDE, 64], [1, FRAME - PW]])
                eng.dma_start(dst[:, 0:FRAME - PW], ap)

            # ================= ConvLSTM layers =================
            with tc.tile_pool(name="state", bufs=1) as state, \
                 tc.tile_pool(name="ps", bufs=4, space="PSUM") as psp:
                hcur = [state.tile([128, FSTRIDE], BF, tag=f"hcur{i}",
                                   name=f"hcur{i}") for i in range(2)]
                for hc in hcur:
                    nc.vector.memset(hc[:], 0.0)
                hB = state.tile([128, FSTRIDE], BF, tag="hB", name="hB")
                c_t = state.tile([128, H * W], F32, tag="c")
                stage = [state.tile([128, FSTRIDE], BF, tag=f"stage{i}",
                                    name=f"stage{i}") for i in range(2)]
                for st in stage:
                    nc.vector.memset(st[:], 0.0)
                ring = state.tile([3 * TT, H * W], BF, tag="ring")
                nc.vector.memset(ring[:], 0.0)

                # piece boundaries aligned to 16 padded rows so that piece p
                # only reads rows already written by back-pairs <= p (a piece
                # issued before its writer would read stale data)
                PBND = [0, 16 * PW, 32 * PW, 48 * PW, FRAME]

                for layer in range(3):
                    g = wgroups[layer]
                    rd_scr = scratch[(layer + 1) % 2]
                    wr_scr = scratch[layer % 2]
                    for t in range(TT):
                        hprev = hcur[(t + 1) % 2]
                        hnew = hcur[t % 2]
                        stg = stage[t % 2]

                        if layer == 0:
                            imt = frp.tile([9, FSTRIDE], BF, tag="im")
                            xap = d_x[:].rearrange("o n -> (o n)")
                            src = bass.AP(xap.tensor,
                                          xap.offset + XLEAD + t * FRAME,
                                          [[PW, 3], [1, 3], [1, FSTRIDE]])
                            nc.gpsimd.dma_start(imt[:], src)
                            inA = inB = None
                        else:
                            inA = frp.tile([128, FSTRIDE], BF, tag="inA")
                            frame_dma_A(inA, rd_scr, t, nc.gpsimd)
                            inB = frp.tile([128, FSTRIDE], BF, tag="inB")
                            frame_dma_B(inB, rd_scr, t, nc.gpsimd)

                        def gate_front(chunk, psA, psB):
                            nc.scalar.activation(psA[:], psA[:],
                                                 AF.Sigmoid,
                                                 bias=b_if[layer][:])
                            g_t = gates.tile([64, CHUNK], F32,
                                             tag="g_t", name="g_t")
                            nc.scalar.activation(
                                g_t[:], psB[0:64, :], AF.Tanh,
                                bias=b_go[layer][0:64, :])
                            nc.scalar.activation(
                                psB[64:128, :], psB[64:128, :],
                                AF.Sigmoid, bias=b_go[layer][64:128, :])
                            # i * tanh(g), written directly onto the
                            # c-chain partitions (cross-partition write)
                            ig = gates.tile([128, CHUNK], F32, tag="ig",
                                            name="ig")
                            nc.vector.tensor_mul(ig[64:128, :],
                                                 psA[0:64, :], g_t[:])
                            return ig

                        def gate_back_pair(items, t, hnew, stg):
                            c0 = items[0][0]
                            n = len(items)
                            # psA-freeing c-multiplies first so the next
                            # timestep's matmuls can reuse the banks early
                            if t == 0:
                                for (cc, psA, psB, ig) in items:
                                    csl = c_t[64:128,
                                              cc * CHUNK:(cc + 1) * CHUNK]
                                    nc.vector.tensor_copy(csl,
                                                          ig[64:128, :])
                            else:
                                for (cc, psA, psB, ig) in items:
                                    csl = c_t[64:128,
                                              cc * CHUNK:(cc + 1) * CHUNK]
                                    nc.vector.tensor_mul(csl, csl,
                                                         psA[64:128, :])
                                for (cc, psA, psB, ig) in items:
                                    csl = c_t[64:128,
                                              cc * CHUNK:(cc + 1) * CHUNK]
                                    nc.vector.tensor_add(csl, csl,
                                                         ig[64:128, :])
                            tc2 = gates.tile([128, 2 * CHUNK], F32,
                                             tag="tc", name="tc")
                            nc.scalar.activation(
                                tc2[64:128, 0:n * CHUNK],
                                c_t[64:128, c0 * CHUNK:(c0 + n) * CHUNK],
                                AF.Tanh)
                            for j, (cc, psA, psB, ig) in enumerate(items):
                                nc.vector.tensor_mul(
                                    interior_ap(hnew, cc),
                                    psB[64:128, :],
                                    tc2[64:128,
                                        j * CHUNK:(j + 1) * CHUNK])
                            nc.scalar.activation(
                                interior_ap(stg, c0, n),
                                interior_ap(hnew, c0, n), AF.Relu,
                                bias=be_t[layer][64:128, :],
                                scale=gb_t[layer][64:128, :])

                        def emit_piece(p):
                            lo, hi = PBND[p], PBND[p + 1]
                            # h replica: lower <- upper shifted +1
                            nc.sync.dma_start(
                                sub_ap(hnew, 0, 64, lo, [[1, hi - lo]]),
                                sub_ap(hnew, 64, 64, lo + 1,
                                       [[1, hi - lo]]))
                            # stage piece out to DRAM scratch
                            nc.sync.dma_start(
                                wr_scr[:][t, :, lo:hi],
                                stg[64:128, lo:hi])

                        def emit_hB(p):
                            # hB: lower <- h data, upper <- h shifted +66.
                            # The +66 source needs one row of the next back
                            # pair, so callers emit piece p after pair p+1.
                            lo, hi = PBND[p], PBND[p + 1]
                            nc.gpsimd.dma_start(
                                sub_ap(hB, 0, 64, lo, [[1, hi - lo]]),
                                sub_ap(hnew, 64, 64, lo, [[1, hi - lo]]))
                            hi_s = min(hi, FRAME - PW)
                            nc.gpsimd.dma_start(
                                sub_ap(hB, 64, 64, lo, [[1, hi_s - lo]]),
                                sub_ap(hnew, 64, 64, lo + PW,
                                       [[1, hi_s - lo]]))

                        pend_pairs = []
                        cur = []
                        done_pieces = 0
                        for chunk in range(NCHUNK):
                            psA = psp.tile([128, CHUNK], F32, tag="psA")
                            psB = psp.tile([128, CHUNK], F32, tag="psB")
                            for m, pst in ((0, psA), (1, psB)):
                                mms = []
                                for (i0, i1, kp, itl, base) in g['inp']:
                                    wi = wti_t[i0 if m == 0 else i1]
                                    if layer == 0:
                                        rhs = conv_rhs(imt, 9, 0, chunk)
                                        mms.append((wi[0:9, :], rhs))
                                    else:
                                        st = inA if itl == 0 else inB
                                        rhs = conv_rhs(st, kp, base,
                                                       chunk)
                                        mms.append((wi[0:kp, :], rhs))
                                if t > 0:
                                    for (i0, i1, kp, itl, base) in \
                                            g['rec']:
                                        wi = wtr_t[i0 if m == 0 else i1]
                                        src = hprev if itl == 0 else hB
                                        rhs = conv_rhs(src, kp, base,
                                                       chunk)
                                        mms.append((wi[0:kp, :], rhs))
                                nmm = len(mms)
                                for j, (lw, rhs) in enumerate(mms):
                                    nc.tensor.matmul(
                                        pst[:], lw, rhs,
                                        start=(j == 0),
                                        stop=(j == nmm - 1))

                            ig = gate_front(chunk, psA, psB)
                            cur.append((chunk, psA, psB, ig))
                            if len(cur) == 2:
                                pend_pairs.append(cur)
                                cur = []
                                if len(pend_pairs) == 2:
                                    items = pend_pairs.pop(0)
                                    gate_back_pair(items, t, hnew, stg)
                                    emit_piece(done_pieces)
                                    if done_pieces >= 1 and t < TT - 1:
                                        emit_hB(done_pieces - 1)
                                    done_pieces += 1

                        for items in pend_pairs:
                            gate_back_pair(items, t, hnew, stg)
                            emit_piece(done_pieces)
                            if t < TT - 1:
                                emit_hB(done_pieces - 1)
                            done_pieces += 1
                        pend_pairs = []
                        if t < TT - 1:
                            emit_hB(3)

                        if DBG:
                            nc.sync.dma_start(d_dbg[layer][:][t, :, :],
                                              stg[64:128, 0:FRAME])

                        if layer == 2:
                            # conv3d contribution of frame t, overlapped
                            # with the recurrence
                            c3A = frp.tile([128, FSTRIDE], BF, tag="inA")
                            frame_dma_A(c3A, wr_scr, t, nc.gpsimd)
                            c3B = frp.tile([128, FSTRIDE], BF, tag="inB")
                            frame_dma_B(c3B, wr_scr, t, nc.gpsimd)
                            pstf = c3s.tile([3, H * W], BF, tag="pstf")
                            n3 = len(w3table)
                            for chunk in range(NCHUNK):
                                pPt = psp.tile([128, CHUNK], F32,
                                               tag="psA")
                                pP = pPt[0:3, :]
                                for j, (wi, kp, itl, base) in \
                                        enumerate(w3table):
                                    st3 = c3A if itl == 0 else c3B
                                    rhs = conv_rhs(st3, kp, base, chunk)
                                    nc.tensor.matmul(
                                        pP, w3tiles[wi][0:kp, 0:3], rhs,
                                        start=(j == 0),
                                        stop=(j == n3 - 1))
                                nc.scalar.activation(
                                    pstf[:,
                                         chunk * CHUNK:
                                         (chunk + 1) * CHUNK],
                                    pP, AF.Copy)
                            # scatter rows m -> ring partition
                            # m*TT + (t+1-m) for valid m
                            ms = [m for m in range(3)
                                  if 0 <= t + 1 - m < TT]
                            m0, mn = ms[0], len(ms)
                            rb = ring[:]
                            dst = bass.AP(rb.tensor,
                                          rb.offset +
                                          (m0 * TT + t + 1 - m0) *
                                          rb.ap[0][0],
                                          [[(TT - 1) * rb.ap[0][0], mn],
                                           [1, H * W]])
                            psrc = pstf[:]
                            srcp = bass.AP(psrc.tensor,
                                           psrc.offset +
                                           m0 * psrc.ap[0][0],
                                           [[psrc.ap[0][0], mn],
                                            [1, H * W]])
                            nc.sync.dma_start(dst, srcp)

                # ======== final: sum shifted planes + sigmoid ========
                for chunk in range(NCHUNK):
                    pYt = psp.tile([128, CHUNK], F32, tag="psA")
                    pY = pYt[0:TT, :]
                    nc.tensor.matmul(
                        pY, bdt[:],
                        ring[:, chunk * CHUNK:(chunk + 1) * CHUNK],
                        start=True, stop=True)
                    ystg = c3s.tile([TT, CHUNK], F32, tag="ystg")
                    nc.scalar.activation(ystg[:], pY, AF.Sigmoid,
                                         bias=b3t[:])
                    nc.sync.dma_start(
                        d_y[:][:, chunk * CHUNK:(chunk + 1) * CHUNK],
                        ystg[:])

    nc.compile()
    return nc


def prep_inputs(x, k0, rk0, b0, g0, be0, k1, rk1, b1, g1, be1,
                k2, rk2, b2, g2, be2, w3, b3, TT=T):
    x = np.asarray(x, np.float32)
    wti, wtr, _ = pack_weights(
        [np.asarray(k0, np.float32), np.asarray(k1, np.float32),
         np.asarray(k2, np.float32)],
        [np.asarray(rk0, np.float32), np.asarray(rk1, np.float32),
         np.asarray(rk2, np.float32)])
    w3t, _ = pack_w3(np.asarray(w3, np.float32))
    b_all = np.stack([np.asarray(b0, np.float32),
                      np.asarray(b1, np.float32),
                      np.asarray(b2, np.float32)])
    scale = np.float32(1.0 / np.sqrt(1.0 + 1e-3))
    gb_all = np.stack([np.asarray(g0, np.float32) * scale,
                       np.asarray(g1, np.float32) * scale,
                       np.asarray(g2, np.float32) * scale])
    be_all = np.stack([np.asarray(be0, np.float32),
                       np.asarray(be1, np.float32),
                       np.asarray(be2, np.float32)])
    bd = np.zeros((3 * TT, TT), np.float32)
    for m in range(3):
        for t in range(TT):
            bd[m * TT + t, t] = 1.0
    b3b = np.full((TT, 1), np.asarray(b3, np.float32).ravel()[0], np.float32)

    shared = dict(wti=wti, wtr=wtr, w3t=w3t, bd=bd.astype(BF16),
                  b_all=b_all, gb_all=gb_all, be_all=be_all, b3b=b3b)
    in_maps = []
    for bb in range(B):
        xi = np.zeros((1, XLEN), BF16)
        fr = np.zeros((TT, PW, PW), np.float32)
        fr[:, 1:H + 1, 1:W + 1] = x[bb, :TT, :, :, 0]
        xi[0, XLEAD:XLEAD + TT * FRAME] = fr.reshape(-1).astype(BF16)
        m = dict(shared)
        m["x_im"] = xi
        in_maps.append(m)
    return in_maps


_CACHED = {}


def kernel(**inputs):
    from concourse.bass_utils import run_bass_kernel_spmd
    if 'nc' not in _CACHED:
        _CACHED['nc'] = build_nc(T)
    nc = _CACHED['nc']
    in_maps = prep_inputs(**inputs)
    res = run_bass_kernel_spmd(nc, in_maps, core_ids=list(range(B)),
                               trace=bool(os.environ.get('KTRACE')))
    _CACHED['last_res'] = res
    y = np.stack([r["y"].reshape(T, H, W, 1) for r in res.results])
    return y


# revision 20
# speedup vs baseline: 1.1432x; 1.1432x over previous
"""Trainium2 Bass kernel for 3-layer ConvLSTM2D stack + BN/ReLU + Conv3D+sigmoid.

Model (per reference):
  for l in 0..2:  h = bn_relu(conv_lstm(h, k_l, rk_l, b_l), g_l, be_l)
  y = sigmoid(conv3d(h, w3) + b3)

Shapes: x [B=8, T=12, 64, 64, 1], F=64 filters per layer, 3x3 kernels, SAME.

Sharding: data-parallel over B across 8 NeuronCores (1 image/core), weights
replicated.

Layout: channels on SBUF partitions, pixels on the free axis in zero-padded
66x66 frames (flat 4356, stride 4360). A 3x3 conv = 9 shifted-tap matmuls
accumulated in PSUM (K=chans, M=out-chans, N=pixels), taps K-packed 2-per-
matmul via shifted replicas on the other 64 partitions of the moving tile:
  tile A = (src, src shifted +1); tile B = (src, src shifted +66)
All matmul operands are bf16 (enables fast weight load); PSUM accumulates
fp32; the c state stays fp32 in SBUF.

Gates: z lives in PSUM as psA=[i;f], psB=[g;o]. c/h reside on partitions
64:128 (same as f,o); the DVE computes i*tanh(g) with a cross-partition
write directly onto 64:128, so no relocation matmul is needed. BN+ReLU is
one ScalarE op (scale=g/sqrt(1+eps), bias=be).

Pipeline: per timestep the 8 pixel chunks are processed in pairs with the
gate math lagging one pair behind the matmuls; h-replica and stage-output
DMAs are issued per quarter-frame piece as soon as their source chunks are
done, so timestep t+1's matmuls start while t's gate tail is still in
flight. Frame loads run on the gpsimd queue, replica/stage pieces on sync,
keeping the scalar queue free for activations.
"""
import sys
import os

_REPO = '/opt/trn_rl_repo'
if _REPO not in sys.path:
    sys.path.insert(0, _REPO)

import numpy as np  # noqa: E402
import ml_dtypes  # noqa: E402

T, H, W, F = 12, 64, 64, 64
B = 8
PW = H + 2                 # padded width = 66
FRAME = PW * PW            # 4356
FSTRIDE = FRAME + 4        # frame stride per channel = 4360
NCHUNK = 8
CROWS = H // NCHUNK        # 8 rows per chunk
CHUNK = CROWS * W          # 512 pixels per chunk
INTER = PW + 1             # interior base offset = 67
XLEAD = PW + 1
XLEN = XLEAD + T * FRAME + 160

PAIRS_A = [((0, 0), (0, 1)), ((1, 0), (1, 1)), ((2, 0), (2, 1))]
PAIR_B = ((0, 2), (1, 2))
SINGLE_A = (2, 2)
REC_SINGLES = [(0, 2), (1, 2), (2, 2)]


def _off(t):
    return (t[0] - 1) * PW + (t[1] - 1)


A_PAIR_BASE = [INTER + _off(p[0]) for p in PAIRS_A]          # 0, 66, 132
B_PAIR_BASE = INTER + _off(PAIR_B[0])                        # 2
A_SINGLE_BASE = INTER + _off(SINGLE_A)                       # 134
REC_SINGLE_BASE = [INTER + _off(t) for t in REC_SINGLES]     # 2, 68, 134

BF16 = ml_dtypes.bfloat16


def pack_weights(ks, rks):
    """Pack conv weights.

    Returns (wt_inp[bf16], wt_rec[bf16], groups); groups[layer] =
    {'inp': [(i0, i1, kp, in_tile, base)], 'rec': [...]}; in_tile 0=A 1=B;
    kp: 128 pair, 64 single (lower), -64 single (upper, h_cur data half).
    """
    ti, tr = [], []
    groups = []

    def addt(lst, w):
        lst.append(w)
        return len(lst) - 1

    def pair(k, ta, tb, m, swap):
        w = np.zeros((128, 128), np.float32)
        lo, hi = (tb, ta) if swap else (ta, tb)
        w[0:64, :] = k[lo[0], lo[1], :, m * 128:(m + 1) * 128]
        w[64:128, :] = k[hi[0], hi[1], :, m * 128:(m + 1) * 128]
        return w

    def single(k, t, m, upper):
        w = np.zeros((128, 128), np.float32)
        r0 = 64 if upper else 0
        w[r0:r0 + 64, :] = k[t[0], t[1], :, m * 128:(m + 1) * 128]
        return w

    for layer in range(3):
        k, rk = ks[layer], rks[layer]
        g = {'inp': [], 'rec': []}
        if layer == 0:
            k9 = k.reshape(9, 4 * F)
            idx = []
            for m in range(2):
                w = np.zeros((128, 128), np.float32)
                w[0:9, :] = k9[:, m * 128:(m + 1) * 128]
                idx.append(addt(ti, w))
            g['inp'].append((idx[0], idx[1], 9, 0, 0))
        else:
            for i, (ta, tb) in enumerate(PAIRS_A):
                g['inp'].append((addt(ti, pair(k, ta, tb, 0, False)),
                                 addt(ti, pair(k, ta, tb, 1, False)),
                                 128, 0, A_PAIR_BASE[i]))
            g['inp'].append((addt(ti, pair(k, PAIR_B[0], PAIR_B[1], 0,
                                           False)),
                             addt(ti, pair(k, PAIR_B[0], PAIR_B[1], 1,
                                           False)),
                             128, 1, B_PAIR_BASE))
            g['inp'].append((addt(ti, single(k, SINGLE_A, 0, False)),
                             addt(ti, single(k, SINGLE_A, 1, False)),
                             64, 0, A_SINGLE_BASE))
        for i, (ta, tb) in enumerate(PAIRS_A):
            g['rec'].append((addt(tr, pair(rk, ta, tb, 0, True)),
                             addt(tr, pair(rk, ta, tb, 1, True)),
                             128, 0, A_PAIR_BASE[i]))
        # B-structure pair and single read the hB tile (h data lower,
        # h shifted +66 upper) — same layout as the input-path tiles
        g['rec'].append((addt(tr, pair(rk, PAIR_B[0], PAIR_B[1], 0, False)),
                         addt(tr, pair(rk, PAIR_B[0], PAIR_B[1], 1, False)),
                         128, 2, B_PAIR_BASE))
        g['rec'].append((addt(tr, single(rk, SINGLE_A, 0, False)),
                         addt(tr, single(rk, SINGLE_A, 1, False)),
                         64, 2, A_SINGLE_BASE))
        groups.append(g)

    return (np.stack(ti).astype(BF16), np.stack(tr).astype(BF16),
            groups)


N_WTI = 22
N_WTR = 30  # 10 * 3


def pack_w3(w3):
    """conv3d weights -> bf16 [5,128,4] tiles (3 cols used) + table."""
    w3 = w3[:, :, :, :, 0]  # [3(dt), 3, 3, 64]
    tiles = np.zeros((5, 128, 4), np.float32)
    table = []
    for i, (ta, tb) in enumerate(PAIRS_A):
        for m in range(3):
            tiles[i, 0:64, m] = w3[m, ta[0], ta[1], :]
            tiles[i, 64:128, m] = w3[m, tb[0], tb[1], :]
        table.append((i, 128, 0, A_PAIR_BASE[i]))
    for m in range(3):
        tiles[3, 0:64, m] = w3[m, PAIR_B[0][0], PAIR_B[0][1], :]
        tiles[3, 64:128, m] = w3[m, PAIR_B[1][0], PAIR_B[1][1], :]
    table.append((3, 128, 1, B_PAIR_BASE))
    for m in range(3):
        tiles[4, 0:64, m] = w3[m, SINGLE_A[0], SINGLE_A[1], :]
    table.append((4, 64, 0, A_SINGLE_BASE))
    return tiles.astype(BF16), table


def build_nc(TT=T):
    import concourse.bass as bass
    import concourse.mybir as mybir
    import concourse.tile as tile
    from concourse import bacc

    F32, F32R, BF = mybir.dt.float32, mybir.dt.float32r, mybir.dt.bfloat16
    AF = mybir.ActivationFunctionType

    nc = bacc.Bacc("TRN2", target_bir_lowering=False, debug=False,
                   num_devices=8)

    d_x = nc.dram_tensor("x_im", [1, XLEN], BF, kind="ExternalInput")
    d_wti = nc.dram_tensor("wti", [N_WTI, 128, 128], BF, kind="ExternalInput")
    d_wtr = nc.dram_tensor("wtr", [N_WTR, 128, 128], BF,
                           kind="ExternalInput")
    d_w3 = nc.dram_tensor("w3t", [5, 128, 4], BF, kind="ExternalInput")
    d_bd = nc.dram_tensor("bd", [3 * TT, TT], BF, kind="ExternalInput")
    d_b = nc.dram_tensor("b_all", [3, 256], F32, kind="ExternalInput")
    d_gb = nc.dram_tensor("gb_all", [3, 64], F32, kind="ExternalInput")
    d_be = nc.dram_tensor("be_all", [3, 64], F32, kind="ExternalInput")
    d_b3 = nc.dram_tensor("b3b", [TT, 1], F32, kind="ExternalInput")
    d_y = nc.dram_tensor("y", [TT, H * W], F32, kind="ExternalOutput")
    DBG = bool(os.environ.get("KDBG"))
    d_dbg = [nc.dram_tensor(f"dbg{l}", [TT, 64, FRAME], BF,
                            kind="ExternalOutput") if DBG else None
             for l in range(3)]

    _, _, wgroups = pack_weights(
        [np.zeros((3, 3, 1, 256), np.float32)] +
        [np.zeros((3, 3, 64, 256), np.float32)] * 2,
        [np.zeros((3, 3, 64, 256), np.float32)] * 3)
    _, w3table = pack_w3(np.zeros((3, 3, 3, 64, 1), np.float32))

    def sub_ap(tile_obj, p0, np_, free_off, free_dims):
        base = tile_obj[:]
        ps = base.ap[0][0]
        return bass.AP(base.tensor, base.offset + p0 * ps + free_off,
                       [[ps, np_]] + [list(d) for d in free_dims])

    def conv_rhs(tile_obj, kparts, base, chunk):
        return sub_ap(tile_obj, 0, kparts, base + chunk * CROWS * PW,
                      [[PW, CROWS], [1, W]])

    def interior_ap(tile_obj, chunk, nchunks=1, p0=64):
        return sub_ap(tile_obj, p0, 64, INTER + chunk * CROWS * PW,
                      [[PW, nchunks * CROWS], [1, W]])

    with tile.TileContext(nc) as tc:
        with tc.tile_pool(name="small", bufs=1) as small, \
             tc.tile_pool(name="gates", bufs=3) as gates, \
             tc.tile_pool(name="c3s", bufs=2) as c3s, \
             tc.tile_pool(name="fr", bufs=3) as frp, \
             tc.tile_pool(name="dram", bufs=1, space="DRAM") as dpool:

            b_if, b_go, gb_t, be_t = [], [], [], []
            for l in range(3):
                tt1 = small.tile([128, 1], F32, tag=f"bif{l}")
                nc.sync.dma_start(
                    tt1[:], d_b[:][l, 0:128].rearrange("(a o) -> a o", o=1))
                b_if.append(tt1)
                tt2 = small.tile([128, 1], F32, tag=f"bgo{l}")
                nc.sync.dma_start(
                    tt2[:], d_b[:][l, 128:256].rearrange("(a o) -> a o", o=1))
                b_go.append(tt2)
                tt4 = small.tile([128, 1], F32, tag=f"gb{l}")
                nc.sync.dma_start(
                    tt4[64:128, :],
                    d_gb[:][l, :].rearrange("(a o) -> a o", o=1))
                gb_t.append(tt4)
                tt5 = small.tile([128, 1], F32, tag=f"be{l}")
                nc.sync.dma_start(
                    tt5[64:128, :],
                    d_be[:][l, :].rearrange("(a o) -> a o", o=1))
                be_t.append(tt5)
            b3t = small.tile([TT, 1], F32, tag="b3")
            nc.sync.dma_start(b3t[:], d_b3[:])

            scratch = [dpool.tile([TT, 64, FSTRIDE], BF, tag=f"scr{i}",
                                  name=f"scr{i}") for i in range(2)]

            wti_t, wtr_t = {}, {}
            for i in range(N_WTI):
                wt_ = small.tile([128, 128], BF, tag=f"wti{i}",
                                 name=f"wti{i}")
                nc.sync.dma_start(wt_[:], d_wti[:][i, :, :])
                wti_t[i] = wt_
            for i in range(N_WTR):
                wt_ = small.tile([128, 128], BF, tag=f"wtr{i}",
                                 name=f"wtr{i}")
                nc.gpsimd.dma_start(wt_[:], d_wtr[:][i, :, :])
                wtr_t[i] = wt_
            w3tiles = []
            for i in range(5):
                w3i = small.tile([128, 4], BF, tag=f"w3_{i}")
                nc.sync.dma_start(w3i[:], d_w3[:][i, :, :])
                w3tiles.append(w3i)
            bdt = small.tile([3 * TT, TT], BF, tag="bd")
            nc.sync.dma_start(bdt[:], d_bd[:])

            def frame_dma_A(dst, scr, t, eng):
                src = scr[:][t, :, :].rearrange("a b -> (a b)")
                ap = bass.AP(src.tensor, src.offset,
                             [[1, 2], [FSTRIDE, 64], [1, FRAME]])
                eng.dma_start(dst[:, 0:FRAME], ap)

            def frame_dma_B(dst, scr, t, eng):
                src = scr[:][t, :, :].rearrange("a b -> (a b)")
                ap = bass.AP(src.tensor, src.offset,
                             [[PW, 2], [FSTRIDE, 64], [1, FRAME - PW]])
                eng.dma_start(dst[:, 0:FRAME - PW], ap)

            # ================= ConvLSTM layers =================
            with tc.tile_pool(name="state", bufs=1) as state, \
                 tc.tile_pool(name="tcp", bufs=2) as tcp, \
                 tc.tile_pool(name="ps", bufs=4, space="PSUM") as psp:
                hcur = [state.tile([128, FSTRIDE], BF, tag=f"hcur{i}",
                                   name=f"hcur{i}") for i in range(2)]
                for hc in hcur:
                    nc.vector.memset(hc[:], 0.0)
                hB = state.tile([128, FSTRIDE], BF, tag="hB", name="hB")
                c_t = state.tile([128, H * W], F32, tag="c")
                stage = [state.tile([128, FSTRIDE], BF, tag=f"stage{i}",
                                    name=f"stage{i}") for i in range(2)]
                for st in stage:
                    nc.vector.memset(st[:], 0.0)
                ring = state.tile([3 * TT, H * W], BF, tag="ring")
                nc.vector.memset(ring[:], 0.0)
                pstf = state.tile([3, H * W], BF, tag="pstf")

                # piece boundaries aligned to 16 padded rows so that piece p
                # only reads rows already written by back-pairs <= p (a piece
                # issued before its writer would read stale data)
                PBND = [0, 16 * PW, 32 * PW, 48 * PW, FRAME]

                def conv3d_block(tp, c3A, c3B):
                    # conv3d contribution of frame tp (frames prefetched
                    # piecewise during the previous timestep)
                    n3 = len(w3table)
                    for chunk in range(NCHUNK):
                        pPt = psp.tile([128, CHUNK], F32, tag="psA")
                        pP = pPt[0:3, :]
                        for j, (wi, kp, itl, base) in enumerate(w3table):
                            st3 = c3A if itl == 0 else c3B
                            rhs = conv_rhs(st3, kp, base, chunk)
                            nc.tensor.matmul(pP, w3tiles[wi][0:kp, 0:3],
                                             rhs, start=(j == 0),
                                             stop=(j == n3 - 1))
                        nc.vector.tensor_copy(
                            pstf[:, chunk * CHUNK:(chunk + 1) * CHUNK],
                            pP)
                    # scatter rows m -> ring partition m*TT + (tp+1-m)
                    ms = [m for m in range(3) if 0 <= tp + 1 - m < TT]
                    m0, mn = ms[0], len(ms)
                    rb = ring[:]
                    dst = bass.AP(rb.tensor,
                                  rb.offset + (m0 * TT + tp + 1 - m0) *
                                  rb.ap[0][0],
                                  [[(TT - 1) * rb.ap[0][0], mn],
                                   [1, H * W]])
                    psrc = pstf[:]
                    srcp = bass.AP(psrc.tensor,
                                   psrc.offset + m0 * psrc.ap[0][0],
                                   [[psrc.ap[0][0], mn], [1, H * W]])
                    nc.sync.dma_start(dst, srcp)

                c3_prev = None
                for layer in range(3):
                    g = wgroups[layer]
                    rd_scr = scratch[(layer + 1) % 2]
                    wr_scr = scratch[layer % 2]
                    for t in range(TT):
                        hprev = hcur[(t + 1) % 2]
                        hnew = hcur[t % 2]
                        stg = stage[t % 2]

                        if layer == 0:
                            imt = c3s.tile([9, FSTRIDE], BF, tag="im")
                            xap = d_x[:].rearrange("o n -> (o n)")
                            src = bass.AP(xap.tensor,
                                          xap.offset + XLEAD + t * FRAME,
                                          [[PW, 3], [1, 3], [1, FSTRIDE]])
                            nc.gpsimd.dma_start(imt[:], src)
                            inA = inB = None
                        else:
                            inA = frp.tile([128, FSTRIDE], BF, tag="inA")
                            frame_dma_A(inA, rd_scr, t, nc.gpsimd)
                            inB = frp.tile([128, FSTRIDE], BF, tag="inB")
                            frame_dma_B(inB, rd_scr, t, nc.gpsimd)

                        if layer == 2 and c3_prev is not None:
                            conv3d_block(*c3_prev)
                            c3_prev = None
                        c3A = c3B = None

                        def gate_front(chunk, psA, psB):
                            nc.scalar.activation(psA[:], psA[:],
                                                 AF.Sigmoid,
                                                 bias=b_if[layer][:])
                            g_t = gates.tile([64, CHUNK], F32,
                                             tag="g_t", name="g_t")
                            nc.scalar.activation(
                                g_t[:], psB[0:64, :], AF.Tanh,
                                bias=b_go[layer][0:64, :])
                            nc.scalar.activation(
                                psB[64:128, :], psB[64:128, :],
                                AF.Sigmoid, bias=b_go[layer][64:128, :])
                            # i * tanh(g), written directly onto the
                            # c-chain partitions (cross-partition write)
                            ig = gates.tile([128, CHUNK], F32, tag="ig",
                                            name="ig")
                            nc.vector.tensor_mul(ig[64:128, :],
                                                 psA[0:64, :], g_t[:])
                            return ig

                        def gate_back_pair(items, t, hnew, stg):
                            c0 = items[0][0]
                            n = len(items)
                            # psA-freeing c-multiplies first so the next
                            # timestep's matmuls can reuse the banks early
                            if t == 0:
                                for (cc, psA, psB, ig) in items:
                                    csl = c_t[64:128,
                                              cc * CHUNK:(cc + 1) * CHUNK]
                                    nc.vector.tensor_copy(csl,
                                                          ig[64:128, :])
                            else:
                                for (cc, psA, psB, ig) in items:
                                    csl = c_t[64:128,
                                              cc * CHUNK:(cc + 1) * CHUNK]
                                    nc.vector.tensor_mul(csl, csl,
                                                         psA[64:128, :])
                                for (cc, psA, psB, ig) in items:
                                    csl = c_t[64:128,
                                              cc * CHUNK:(cc + 1) * CHUNK]
                                    nc.vector.tensor_add(csl, csl,
                                                         ig[64:128, :])
                            tc2 = tcp.tile([128, 2 * CHUNK], F32,
                                           tag="tc", name="tc")
                            nc.scalar.activation(
                                tc2[64:128, 0:n * CHUNK],
                                c_t[64:128, c0 * CHUNK:(c0 + n) * CHUNK],
                                AF.Tanh)
                            for j, (cc, psA, psB, ig) in enumerate(items):
                                nc.vector.tensor_mul(
                                    interior_ap(hnew, cc),
                                    psB[64:128, :],
                                    tc2[64:128,
                                        j * CHUNK:(j + 1) * CHUNK])
                            nc.scalar.activation(
                                interior_ap(stg, c0, n),
                                interior_ap(hnew, c0, n), AF.Relu,
                                bias=be_t[layer][64:128, :],
                                scale=gb_t[layer][64:128, :])

                        def emit_piece(p):
                            lo, hi = PBND[p], PBND[p + 1]
                            # h replica: lower <- upper shifted +1
                            nc.sync.dma_start(
                                sub_ap(hnew, 0, 64, lo, [[1, hi - lo]]),
                                sub_ap(hnew, 64, 64, lo + 1,
                                       [[1, hi - lo]]))
                            # stage piece out to DRAM scratch
                            nc.sync.dma_start(
                                wr_scr[:][t, :, lo:hi],
                                stg[64:128, lo:hi])

                        def emit_hB(p):
                            # hB: lower <- h data, upper <- h shifted +66.
                            # The +66 source needs one row of the next back
                            # pair, so callers emit piece p after pair p+1.
                            lo, hi = PBND[p], PBND[p + 1]
                            nc.sync.dma_start(
                                sub_ap(hB, 0, 64, lo, [[1, hi - lo]]),
                                sub_ap(hnew, 64, 64, lo, [[1, hi - lo]]))
                            hi_s = min(hi, FRAME - PW)
                            nc.sync.dma_start(
                                sub_ap(hB, 64, 64, lo, [[1, hi_s - lo]]),
                                sub_ap(hnew, 64, 64, lo + PW,
                                       [[1, hi_s - lo]]))

                        def emit_c3(p):
                            # prefetch conv3d frame pieces for this t from
                            # the just-written scratch rows (stage piece
                            # p+1 provides the one-pixel lookahead)
                            lo, hi = PBND[p], PBND[p + 1]
                            srcf = wr_scr[:][t, :, :].rearrange(
                                "a b -> (a b)")
                            ap = bass.AP(srcf.tensor, srcf.offset + lo,
                                         [[1, 2], [FSTRIDE, 64],
                                          [1, hi - lo]])
                            nc.sync.dma_start(c3A[:, lo:hi], ap)
                            hi_b = min(hi, FRAME - PW)
                            if hi_b > lo:
                                ap = bass.AP(srcf.tensor,
                                             srcf.offset + lo,
                                             [[PW, 2], [FSTRIDE, 64],
                                              [1, hi_b - lo]])
                                nc.sync.dma_start(c3B[:, lo:hi_b], ap)

                        pend_pairs = []
                        cur = []
                        done_pieces = 0
                        for chunk in range(NCHUNK):
                            psA = psp.tile([128, CHUNK], F32, tag="psA")
                            psB = psp.tile([128, CHUNK], F32, tag="psB")
                            for m, pst in ((0, psA), (1, psB)):
                                mms = []
                                for (i0, i1, kp, itl, base) in g['inp']:
                                    wi = wti_t[i0 if m == 0 else i1]
                                    if layer == 0:
                                        rhs = conv_rhs(imt, 9, 0, chunk)
                                        mms.append((wi[0:9, :], rhs))
                                    else:
                                        st = inA if itl == 0 else inB
                                        rhs = conv_rhs(st, kp, base,
                                                       chunk)
                                        mms.append((wi[0:kp, :], rhs))
                                if t > 0:
                                    for (i0, i1, kp, itl, base) in \
                                            g['rec']:
                                        wi = wtr_t[i0 if m == 0 else i1]
                                        src = hprev if itl == 0 else hB
                                        rhs = conv_rhs(src, kp, base,
                                                       chunk)
                                        mms.append((wi[0:kp, :], rhs))
                                nmm = len(mms)
                                for j, (lw, rhs) in enumerate(mms):
                                    nc.tensor.matmul(
                                        pst[:], lw, rhs,
                                        start=(j == 0),
                                        stop=(j == nmm - 1))

                            ig = gate_front(chunk, psA, psB)
                            cur.append((chunk, psA, psB, ig))
                            if len(cur) == 2:
                                pend_pairs.append(cur)
                                cur = []
                                if len(pend_pairs) == 2:
                                    items = pend_pairs.pop(0)
                                    gate_back_pair(items, t, hnew, stg)
                                    emit_piece(done_pieces)
                                    if done_pieces >= 1:
                                        if t < TT - 1:
                                            emit_hB(done_pieces - 1)
                                        if layer == 2:
                                            if c3A is None:
                                                c3A = frp.tile(
                                                    [128, FSTRIDE], BF,
                                                    tag="inA")
                                                c3B = frp.tile(
                                                    [128, FSTRIDE], BF,
                                                    tag="inB")
                                            emit_c3(done_pieces - 1)
                                    done_pieces += 1

                        for items in pend_pairs:
                            gate_back_pair(items, t, hnew, stg)
                            emit_piece(done_pieces)
                            if t < TT - 1:
                                emit_hB(done_pieces - 1)
                            if layer == 2:
                                emit_c3(done_pieces - 1)
                            done_pieces += 1
                        pend_pairs = []
                        if t < TT - 1:
                            emit_hB(3)
                        if layer == 2:
                            emit_c3(3)
                            c3_prev = (t, c3A, c3B)

                        if DBG:
                            nc.sync.dma_start(d_dbg[layer][:][t, :, :],
                                              stg[64:128, 0:FRAME])

                # last frame's conv3d, then sum shifted planes + sigmoid
                conv3d_block(*c3_prev)
                for chunk in range(NCHUNK):
                    pYt = psp.tile([128, CHUNK], F32, tag="psA")
                    pY = pYt[0:TT, :]
                    nc.tensor.matmul(
                        pY, bdt[:],
                        ring[:, chunk * CHUNK:(chunk + 1) * CHUNK],
                        start=True, stop=True)
                    ystg = c3s.tile([TT, CHUNK], F32, tag="ystg")
                    nc.scalar.activation(ystg[:], pY, AF.Sigmoid,
                                         bias=b3t[:])
                    nc.sync.dma_start(
                        d_y[:][:, chunk * CHUNK:(chunk + 1) * CHUNK],
                        ystg[:])

    nc.compile()
    return nc


def prep_inputs(x, k0, rk0, b0, g0, be0, k1, rk1, b1, g1, be1,
                k2, rk2, b2, g2, be2, w3, b3, TT=T):
    x = np.asarray(x, np.float32)
    wti, wtr, _ = pack_weights(
        [np.asarray(k0, np.float32), np.asarray(k1, np.float32),
         np.asarray(k2, np.float32)],
        [np.asarray(rk0, np.float32), np.asarray(rk1, np.float32),
         np.asarray(rk2, np.float32)])
    w3t, _ = pack_w3(np.asarray(w3, np.float32))
    b_all = np.stack([np.asarray(b0, np.float32),
                      np.asarray(b1, np.float32),
                      np.asarray(b2, np.float32)])
    scale = np.float32(1.0 / np.sqrt(1.0 + 1e-3))
    gb_all = np.stack([np.asarray(g0, np.float32) * scale,
                       np.asarray(g1, np.float32) * scale,
                       np.asarray(g2, np.float32) * scale])
    be_all = np.stack([np.asarray(be0, np.float32),
                       np.asarray(be1, np.float32),
                       np.asarray(be2, np.float32)])
    bd = np.zeros((3 * TT, TT), np.float32)
    for m in range(3):
        for t in range(TT):
            bd[m * TT + t, t] = 1.0
    b3b = np.full((TT, 1), np.asarray(b3, np.float32).ravel()[0], np.float32)

    shared = dict(wti=wti, wtr=wtr, w3t=w3t, bd=bd.astype(BF16),
                  b_all=b_all, gb_all=gb_all, be_all=be_all, b3b=b3b)
    in_maps = []
    for bb in range(B):
        xi = np.zeros((1, XLEN), BF16)
        fr = np.zeros((TT, PW, PW), np.float32)
        fr[:, 1:H + 1, 1:W + 1] = x[bb, :TT, :, :, 0]
        xi[0, XLEAD:XLEAD + TT * FRAME] = fr.reshape(-1).astype(BF16)
        m = dict(shared)
        m["x_im"] = xi
        in_maps.append(m)
    return in_maps


_CACHED = {}


def kernel(**inputs):
    from concourse.bass_utils import run_bass_kernel_spmd
    if 'nc' not in _CACHED:
        _CACHED['nc'] = build_nc(T)
    nc = _CACHED['nc']
    in_maps = prep_inputs(**inputs)
    res = run_bass_kernel_spmd(nc, in_maps, core_ids=list(range(B)),
                               trace=bool(os.environ.get('KTRACE')))
    _CACHED['last_res'] = res
    y = np.stack([r["y"].reshape(T, H, W, 1) for r in res.results])
    return y


# revision 21
# speedup vs baseline: 1.2431x; 1.0873x over previous
"""Trainium2 Bass kernel for 3-layer ConvLSTM2D stack + BN/ReLU + Conv3D+sigmoid.

Model (per reference):
  for l in 0..2:  h = bn_relu(conv_lstm(h, k_l, rk_l, b_l), g_l, be_l)
  y = sigmoid(conv3d(h, w3) + b3)

Shapes: x [B=8, T=12, 64, 64, 1], F=64 filters per layer, 3x3 kernels, SAME.

Sharding: data-parallel over B across 8 NeuronCores (1 image/core), weights
replicated.

Layout: channels on SBUF partitions, pixels on the free axis in zero-padded
66x66 frames (flat 4356, stride 4360). A 3x3 conv = 9 shifted-tap matmuls
accumulated in PSUM (K=chans, M=out-chans, N=pixels), taps K-packed 2-per-
matmul via shifted replicas on the other 64 partitions of the moving tile:
  tile A = (src, src shifted +1); tile B = (src, src shifted +66)
All matmul operands are bf16 (enables fast weight load); PSUM accumulates
fp32; the c state stays fp32 in SBUF.

Gates: z lives in PSUM as psA=[i;f], psB=[g;o]. c/h reside on partitions
64:128 (same as f,o); the DVE computes i*tanh(g) with a cross-partition
write directly onto 64:128, so no relocation matmul is needed. BN+ReLU is
one ScalarE op (scale=g/sqrt(1+eps), bias=be).

Pipeline: per timestep the 8 pixel chunks are processed in pairs with the
gate math lagging one pair behind the matmuls; h-replica and stage-output
DMAs are issued per quarter-frame piece as soon as their source chunks are
done, so timestep t+1's matmuls start while t's gate tail is still in
flight. Frame loads run on the gpsimd queue, replica/stage pieces on sync,
keeping the scalar queue free for activations.
"""
import sys
import os

_REPO = '/opt/trn_rl_repo'
if _REPO not in sys.path:
    sys.path.insert(0, _REPO)

import numpy as np  # noqa: E402
import ml_dtypes  # noqa: E402

T, H, W, F = 12, 64, 64, 64
B = 8
PW = H + 2                 # padded width = 66
FRAME = PW * PW            # 4356
FSTRIDE = FRAME + 4        # frame stride per channel = 4360
NCHUNK = 8
CROWS = H // NCHUNK        # 8 rows per chunk
CHUNK = CROWS * W          # 512 pixels per chunk
INTER = PW + 1             # interior base offset = 67
XLEAD = PW + 1
XLEN = XLEAD + T * FRAME + 160

PAIRS_A = [((0, 0), (0, 1)), ((1, 0), (1, 1)), ((2, 0), (2, 1))]
PAIR_B = ((0, 2), (1, 2))
SINGLE_A = (2, 2)
REC_SINGLES = [(0, 2), (1, 2), (2, 2)]


def _off(t):
    return (t[0] - 1) * PW + (t[1] - 1)


A_PAIR_BASE = [INTER + _off(p[0]) for p in PAIRS_A]          # 0, 66, 132
B_PAIR_BASE = INTER + _off(PAIR_B[0])                        # 2
A_SINGLE_BASE = INTER + _off(SINGLE_A)                       # 134
REC_SINGLE_BASE = [INTER + _off(t) for t in REC_SINGLES]     # 2, 68, 134

BF16 = ml_dtypes.bfloat16


def pack_weights(ks, rks):
    """Pack conv weights.

    Returns (wt_inp[bf16], wt_rec[bf16], groups); groups[layer] =
    {'inp': [(i0, i1, kp, in_tile, base)], 'rec': [...]}; in_tile 0=A 1=B;
    kp: 128 pair, 64 single (lower), -64 single (upper, h_cur data half).
    """
    ti, tr = [], []
    groups = []

    def addt(lst, w):
        lst.append(w)
        return len(lst) - 1

    def pair(k, ta, tb, m, swap):
        w = np.zeros((128, 128), np.float32)
        lo, hi = (tb, ta) if swap else (ta, tb)
        w[0:64, :] = k[lo[0], lo[1], :, m * 128:(m + 1) * 128]
        w[64:128, :] = k[hi[0], hi[1], :, m * 128:(m + 1) * 128]
        return w

    def single(k, t, m, upper):
        w = np.zeros((128, 128), np.float32)
        r0 = 64 if upper else 0
        w[r0:r0 + 64, :] = k[t[0], t[1], :, m * 128:(m + 1) * 128]
        return w

    for layer in range(3):
        k, rk = ks[layer], rks[layer]
        g = {'inp': [], 'rec': []}
        if layer == 0:
            k9 = k.reshape(9, 4 * F)
            idx = []
            for m in range(2):
                w = np.zeros((128, 128), np.float32)
                w[0:9, :] = k9[:, m * 128:(m + 1) * 128]
                idx.append(addt(ti, w))
            g['inp'].append((idx[0], idx[1], 9, 0, 0))
        else:
            for i, (ta, tb) in enumerate(PAIRS_A):
                g['inp'].append((addt(ti, pair(k, ta, tb, 0, False)),
                                 addt(ti, pair(k, ta, tb, 1, False)),
                                 128, 0, A_PAIR_BASE[i]))
            g['inp'].append((addt(ti, pair(k, PAIR_B[0], PAIR_B[1], 0,
                                           False)),
                             addt(ti, pair(k, PAIR_B[0], PAIR_B[1], 1,
                                           False)),
                             128, 1, B_PAIR_BASE))
            g['inp'].append((addt(ti, single(k, SINGLE_A, 0, False)),
                             addt(ti, single(k, SINGLE_A, 1, False)),
                             64, 0, A_SINGLE_BASE))
        for i, (ta, tb) in enumerate(PAIRS_A):
            g['rec'].append((addt(tr, pair(rk, ta, tb, 0, True)),
                             addt(tr, pair(rk, ta, tb, 1, True)),
                             128, 0, A_PAIR_BASE[i]))
        # B-structure pair and single read the hB tile (h data lower,
        # h shifted +66 upper) — same layout as the input-path tiles
        g['rec'].append((addt(tr, pair(rk, PAIR_B[0], PAIR_B[1], 0, False)),
                         addt(tr, pair(rk, PAIR_B[0], PAIR_B[1], 1, False)),
                         128, 2, B_PAIR_BASE))
        g['rec'].append((addt(tr, single(rk, SINGLE_A, 0, False)),
                         addt(tr, single(rk, SINGLE_A, 1, False)),
                         64, 2, A_SINGLE_BASE))
        groups.append(g)

    return (np.stack(ti).astype(BF16), np.stack(tr).astype(BF16),
            groups)


N_WTI = 22
N_WTR = 30  # 10 * 3


def pack_w3(w3):
    """conv3d weights -> bf16 [5,128,4] tiles (3 cols used) + table."""
    w3 = w3[:, :, :, :, 0]  # [3(dt), 3, 3, 64]
    tiles = np.zeros((5, 128, 4), np.float32)
    table = []
    for i, (ta, tb) in enumerate(PAIRS_A):
        for m in range(3):
            tiles[i, 0:64, m] = w3[m, ta[0], ta[1], :]
            tiles[i, 64:128, m] = w3[m, tb[0], tb[1], :]
        table.append((i, 128, 0, A_PAIR_BASE[i]))
    for m in range(3):
        tiles[3, 0:64, m] = w3[m, PAIR_B[0][0], PAIR_B[0][1], :]
        tiles[3, 64:128, m] = w3[m, PAIR_B[1][0], PAIR_B[1][1], :]
    table.append((3, 128, 1, B_PAIR_BASE))
    for m in range(3):
        tiles[4, 0:64, m] = w3[m, SINGLE_A[0], SINGLE_A[1], :]
    table.append((4, 64, 0, A_SINGLE_BASE))
    return tiles.astype(BF16), table


def build_nc(TT=T):
    import concourse.bass as bass
    import concourse.mybir as mybir
    import concourse.tile as tile
    from concourse import bacc

    F32, F32R, BF = mybir.dt.float32, mybir.dt.float32r, mybir.dt.bfloat16
    AF = mybir.ActivationFunctionType

    nc = bacc.Bacc("TRN2", target_bir_lowering=False, debug=False,
                   num_devices=8)

    d_x = nc.dram_tensor("x_im", [1, XLEN], BF, kind="ExternalInput")
    d_wti = nc.dram_tensor("wti", [N_WTI, 128, 128], BF, kind="ExternalInput")
    d_wtr = nc.dram_tensor("wtr", [N_WTR, 128, 128], BF,
                           kind="ExternalInput")
    d_w3 = nc.dram_tensor("w3t", [5, 128, 4], BF, kind="ExternalInput")
    d_bd = nc.dram_tensor("bd", [3 * TT, TT], BF, kind="ExternalInput")
    d_b = nc.dram_tensor("b_all", [3, 256], F32, kind="ExternalInput")
    d_gb = nc.dram_tensor("gb_all", [3, 64], F32, kind="ExternalInput")
    d_be = nc.dram_tensor("be_all", [3, 64], F32, kind="ExternalInput")
    d_b3 = nc.dram_tensor("b3b", [TT, 1], F32, kind="ExternalInput")
    d_y = nc.dram_tensor("y", [TT, H * W], F32, kind="ExternalOutput")
    DBG = bool(os.environ.get("KDBG"))
    d_dbg = [nc.dram_tensor(f"dbg{l}", [TT, 64, FRAME], BF,
                            kind="ExternalOutput") if DBG else None
             for l in range(3)]

    _, _, wgroups = pack_weights(
        [np.zeros((3, 3, 1, 256), np.float32)] +
        [np.zeros((3, 3, 64, 256), np.float32)] * 2,
        [np.zeros((3, 3, 64, 256), np.float32)] * 3)
    _, w3table = pack_w3(np.zeros((3, 3, 3, 64, 1), np.float32))

    def sub_ap(tile_obj, p0, np_, free_off, free_dims):
        base = tile_obj[:]
        ps = base.ap[0][0]
        return bass.AP(base.tensor, base.offset + p0 * ps + free_off,
                       [[ps, np_]] + [list(d) for d in free_dims])

    def conv_rhs(tile_obj, kparts, base, chunk):
        return sub_ap(tile_obj, 0, kparts, base + chunk * CROWS * PW,
                      [[PW, CROWS], [1, W]])

    def interior_ap(tile_obj, chunk, nchunks=1, p0=64):
        return sub_ap(tile_obj, p0, 64, INTER + chunk * CROWS * PW,
                      [[PW, nchunks * CROWS], [1, W]])

    with tile.TileContext(nc) as tc:
        with tc.tile_pool(name="small", bufs=1) as small, \
             tc.tile_pool(name="gates", bufs=3) as gates, \
             tc.tile_pool(name="c3s", bufs=2) as c3s, \
             tc.tile_pool(name="fr", bufs=3) as frp, \
             tc.tile_pool(name="dram", bufs=1, space="DRAM") as dpool:

            b_if, b_go, gb_t, be_t = [], [], [], []
            for l in range(3):
                tt1 = small.tile([128, 1], F32, tag=f"bif{l}")
                nc.sync.dma_start(
                    tt1[:], d_b[:][l, 0:128].rearrange("(a o) -> a o", o=1))
                b_if.append(tt1)
                tt2 = small.tile([128, 1], F32, tag=f"bgo{l}")
                nc.sync.dma_start(
                    tt2[:], d_b[:][l, 128:256].rearrange("(a o) -> a o", o=1))
                b_go.append(tt2)
                tt4 = small.tile([128, 1], F32, tag=f"gb{l}")
                nc.sync.dma_start(
                    tt4[64:128, :],
                    d_gb[:][l, :].rearrange("(a o) -> a o", o=1))
                gb_t.append(tt4)
                tt5 = small.tile([128, 1], F32, tag=f"be{l}")
                nc.sync.dma_start(
                    tt5[64:128, :],
                    d_be[:][l, :].rearrange("(a o) -> a o", o=1))
                be_t.append(tt5)
            b3t = small.tile([TT, 1], F32, tag="b3")
            nc.sync.dma_start(b3t[:], d_b3[:])

            scratch = [dpool.tile([TT, 64, FSTRIDE], BF, tag=f"scr{i}",
                                  name=f"scr{i}") for i in range(2)]

            wti_t, wtr_t = {}, {}
            for i in range(N_WTI):
                wt_ = small.tile([128, 128], BF, tag=f"wti{i}",
                                 name=f"wti{i}")
                nc.sync.dma_start(wt_[:], d_wti[:][i, :, :])
                wti_t[i] = wt_
            for i in range(N_WTR):
                wt_ = small.tile([128, 128], BF, tag=f"wtr{i}",
                                 name=f"wtr{i}")
                nc.gpsimd.dma_start(wt_[:], d_wtr[:][i, :, :])
                wtr_t[i] = wt_
            w3tiles = []
            for i in range(5):
                w3i = small.tile([128, 4], BF, tag=f"w3_{i}")
                nc.sync.dma_start(w3i[:], d_w3[:][i, :, :])
                w3tiles.append(w3i)
            bdt = small.tile([3 * TT, TT], BF, tag="bd")
            nc.sync.dma_start(bdt[:], d_bd[:])

            def frame_dma_A(dst, scr, t, eng):
                src = scr[:][t, :, :].rearrange("a b -> (a b)")
                ap = bass.AP(src.tensor, src.offset,
                             [[1, 2], [FSTRIDE, 64], [1, FRAME]])
                eng.dma_start(dst[:, 0:FRAME], ap)

            def frame_dma_B(dst, scr, t, eng):
                src = scr[:][t, :, :].rearrange("a b -> (a b)")
                ap = bass.AP(src.tensor, src.offset,
                             [[PW, 2], [FSTRIDE, 64], [1, FRAME - PW]])
                eng.dma_start(dst[:, 0:FRAME - PW], ap)

            # ================= ConvLSTM layers =================
            with tc.tile_pool(name="state", bufs=1) as state, \
                 tc.tile_pool(name="tcp", bufs=2) as tcp, \
                 tc.tile_pool(name="ps", bufs=4, space="PSUM") as psp:
                hcur = [state.tile([128, FSTRIDE], BF, tag=f"hcur{i}",
                                   name=f"hcur{i}") for i in range(2)]
                for hc in hcur:
                    nc.vector.memset(hc[:], 0.0)
                hB = state.tile([128, FSTRIDE], BF, tag="hB", name="hB")
                c_t = state.tile([128, H * W], F32, tag="c")
                stage = [state.tile([128, FSTRIDE], BF, tag=f"stage{i}",
                                    name=f"stage{i}") for i in range(2)]
                for st in stage:
                    nc.vector.memset(st[:], 0.0)
                ring = state.tile([3 * TT, H * W], BF, tag="ring")
                nc.vector.memset(ring[:], 0.0)
                pstf = state.tile([3, H * W], BF, tag="pstf")

                # piece boundaries aligned to 16 padded rows so that piece p
                # only reads rows already written by back-pairs <= p (a piece
                # issued before its writer would read stale data)
                PBND = [0, 16 * PW, 32 * PW, 48 * PW, FRAME]

                def conv3d_block(tp, c3A, c3B):
                    # conv3d contribution of frame tp (frames prefetched
                    # piecewise during the previous timestep)
                    n3 = len(w3table)
                    for chunk in range(NCHUNK):
                        pPt = psp.tile([128, CHUNK], F32, tag="psA")
                        pP = pPt[0:3, :]
                        for j, (wi, kp, itl, base) in enumerate(w3table):
                            st3 = c3A if itl == 0 else c3B
                            rhs = conv_rhs(st3, kp, base, chunk)
                            nc.tensor.matmul(pP, w3tiles[wi][0:kp, 0:3],
                                             rhs, start=(j == 0),
                                             stop=(j == n3 - 1))
                        nc.vector.tensor_copy(
                            pstf[:, chunk * CHUNK:(chunk + 1) * CHUNK],
                            pP)
                    # scatter rows m -> ring partition m*TT + (tp+1-m)
                    ms = [m for m in range(3) if 0 <= tp + 1 - m < TT]
                    m0, mn = ms[0], len(ms)
                    rb = ring[:]
                    dst = bass.AP(rb.tensor,
                                  rb.offset + (m0 * TT + tp + 1 - m0) *
                                  rb.ap[0][0],
                                  [[(TT - 1) * rb.ap[0][0], mn],
                                   [1, H * W]])
                    psrc = pstf[:]
                    srcp = bass.AP(psrc.tensor,
                                   psrc.offset + m0 * psrc.ap[0][0],
                                   [[psrc.ap[0][0], mn], [1, H * W]])
                    nc.sync.dma_start(dst, srcp)

                c3_prev = None
                for layer in range(3):
                    g = wgroups[layer]
                    rd_scr = scratch[(layer + 1) % 2]
                    wr_scr = scratch[layer % 2]
                    for t in range(TT):
                        hprev = hcur[(t + 1) % 2]
                        hnew = hcur[t % 2]
                        stg = stage[t % 2]

                        if layer == 0:
                            imt = c3s.tile([9, FSTRIDE], BF, tag="im")
                            xap = d_x[:].rearrange("o n -> (o n)")
                            src = bass.AP(xap.tensor,
                                          xap.offset + XLEAD + t * FRAME,
                                          [[PW, 3], [1, 3], [1, FSTRIDE]])
                            nc.gpsimd.dma_start(imt[:], src)
                            inA = inB = None
                        else:
                            inA = frp.tile([128, FSTRIDE], BF, tag="inA")
                            frame_dma_A(inA, rd_scr, t, nc.gpsimd)
                            inB = frp.tile([128, FSTRIDE], BF, tag="inB")
                            frame_dma_B(inB, rd_scr, t, nc.gpsimd)

                        if layer == 2 and c3_prev is not None:
                            conv3d_block(*c3_prev)
                            c3_prev = None
                        c3A = c3B = None

                        def gate_front(chunk, psA, psB):
                            nc.scalar.activation(psA[:], psA[:],
                                                 AF.Sigmoid,
                                                 bias=b_if[layer][:])
                            g_t = gates.tile([64, CHUNK], F32,
                                             tag="g_t", name="g_t")
                            nc.scalar.activation(
                                g_t[:], psB[0:64, :], AF.Tanh,
                                bias=b_go[layer][0:64, :])
                            nc.scalar.activation(
                                psB[64:128, :], psB[64:128, :],
                                AF.Sigmoid, bias=b_go[layer][64:128, :])
                            # i * tanh(g), written directly onto the
                            # c-chain partitions (cross-partition write)
                            ig = gates.tile([128, CHUNK], F32, tag="ig",
                                            name="ig")
                            nc.vector.tensor_mul(ig[64:128, :],
                                                 psA[0:64, :], g_t[:])
                            return ig

                        def gate_back_pair(items, t, hnew, stg):
                            c0 = items[0][0]
                            n = len(items)
                            # psA-freeing c-multiplies first so the next
                            # timestep's matmuls can reuse the banks early
                            if t == 0:
                                for (cc, psA, psB, ig) in items:
                                    csl = c_t[64:128,
                                              cc * CHUNK:(cc + 1) * CHUNK]
                                    nc.vector.tensor_copy(csl,
                                                          ig[64:128, :])
                            else:
                                for (cc, psA, psB, ig) in items:
                                    csl = c_t[64:128,
                                              cc * CHUNK:(cc + 1) * CHUNK]
                                    nc.vector.tensor_mul(csl, csl,
                                                         psA[64:128, :])
                                for (cc, psA, psB, ig) in items:
                                    csl = c_t[64:128,
                                              cc * CHUNK:(cc + 1) * CHUNK]
                                    nc.vector.tensor_add(csl, csl,
                                                         ig[64:128, :])
                            tc2 = tcp.tile([128, 2 * CHUNK], F32,
                                           tag="tc", name="tc")
                            nc.scalar.activation(
                                tc2[64:128, 0:n * CHUNK],
                                c_t[64:128, c0 * CHUNK:(c0 + n) * CHUNK],
                                AF.Tanh)
                            for j, (cc, psA, psB, ig) in enumerate(items):
                                nc.vector.tensor_mul(
                                    interior_ap(hnew, cc),
                                    psB[64:128, :],
                                    tc2[64:128,
                                        j * CHUNK:(j + 1) * CHUNK])
                            nc.scalar.activation(
                                interior_ap(stg, c0, n),
                                interior_ap(hnew, c0, n), AF.Relu,
                                bias=be_t[layer][64:128, :],
                                scale=gb_t[layer][64:128, :])

                        def emit_piece(p):
                            lo, hi = PBND[p], PBND[p + 1]
                            # h replica: lower <- upper shifted +1
                            nc.sync.dma_start(
                                sub_ap(hnew, 0, 64, lo, [[1, hi - lo]]),
                                sub_ap(hnew, 64, 64, lo + 1,
                                       [[1, hi - lo]]))
                            # stage piece out to DRAM scratch
                            nc.sync.dma_start(
                                wr_scr[:][t, :, lo:hi],
                                stg[64:128, lo:hi])

                        def emit_hB(p):
                            # hB: lower <- h data, upper <- h shifted +66.
                            # The +66 source needs one row of the next back
                            # pair, so callers emit piece p after pair p+1.
                            lo, hi = PBND[p], PBND[p + 1]
                            nc.sync.dma_start(
                                sub_ap(hB, 0, 64, lo, [[1, hi - lo]]),
                                sub_ap(hnew, 64, 64, lo, [[1, hi - lo]]))
                            hi_s = min(hi, FRAME - PW)
                            nc.sync.dma_start(
                                sub_ap(hB, 64, 64, lo, [[1, hi_s - lo]]),
                                sub_ap(hnew, 64, 64, lo + PW,
                                       [[1, hi_s - lo]]))

                        def emit_c3(p):
                            # prefetch conv3d frame pieces for this t from
                            # the just-written scratch rows (stage piece
                            # p+1 provides the one-pixel lookahead)
                            lo, hi = PBND[p], PBND[p + 1]
                            srcf = wr_scr[:][t, :, :].rearrange(
                                "a b -> (a b)")
                            ap = bass.AP(srcf.tensor, srcf.offset + lo,
                                         [[1, 2], [FSTRIDE, 64],
                                          [1, hi - lo]])
                            nc.gpsimd.dma_start(c3A[:, lo:hi], ap)
                            hi_b = min(hi, FRAME - PW)
                            if hi_b > lo:
                                ap = bass.AP(srcf.tensor,
                                             srcf.offset + lo,
                                             [[PW, 2], [FSTRIDE, 64],
                                              [1, hi_b - lo]])
                                nc.gpsimd.dma_start(c3B[:, lo:hi_b], ap)

                        pend_pairs = []
                        cur = []
                        done_pieces = 0
                        for chunk in range(NCHUNK):
                            psA = psp.tile([128, CHUNK], F32, tag="psA")
                            psB = psp.tile([128, CHUNK], F32, tag="psB")
                            for m, pst in ((0, psA), (1, psB)):
                                mms = []
                                for (i0, i1, kp, itl, base) in g['inp']:
                                    wi = wti_t[i0 if m == 0 else i1]
                                    if layer == 0:
                                        rhs = conv_rhs(imt, 9, 0, chunk)
                                        mms.append((wi[0:9, :], rhs))
                                    else:
                                        st = inA if itl == 0 else inB
                                        rhs = conv_rhs(st, kp, base,
                                                       chunk)
                                        mms.append((wi[0:kp, :], rhs))
                                if t > 0:
                                    for (i0, i1, kp, itl, base) in \
                                            g['rec']:
                                        wi = wtr_t[i0 if m == 0 else i1]
                                        src = hprev if itl == 0 else hB
                                        rhs = conv_rhs(src, kp, base,
                                                       chunk)
                                        mms.append((wi[0:kp, :], rhs))
                                nmm = len(mms)
                                for j, (lw, rhs) in enumerate(mms):
                                    nc.tensor.matmul(
                                        pst[:], lw, rhs,
                                        start=(j == 0),
                                        stop=(j == nmm - 1))

                            ig = gate_front(chunk, psA, psB)
                            cur.append((chunk, psA, psB, ig))
                            if len(cur) == 2:
                                pend_pairs.append(cur)
                                cur = []
                                if len(pend_pairs) == 2:
                                    items = pend_pairs.pop(0)
                                    gate_back_pair(items, t, hnew, stg)
                                    emit_piece(done_pieces)
                                    if done_pieces >= 1:
                                        if t < TT - 1:
                                            emit_hB(done_pieces - 1)
                                        if layer == 2:
                                            if c3A is None:
                                                c3A = frp.tile(
                                                    [128, FSTRIDE], BF,
                                                    tag="inA")
                                                c3B = frp.tile(
                                                    [128, FSTRIDE], BF,
                                                    tag="inB")
                                            emit_c3(done_pieces - 1)
                                    done_pieces += 1

                        for items in pend_pairs:
                            gate_back_pair(items, t, hnew, stg)
                            emit_piece(done_pieces)
                            if t < TT - 1:
                                emit_hB(done_pieces - 1)
                            if layer == 2:
                                emit_c3(done_pieces - 1)
                            done_pieces += 1
                        pend_pairs = []
                        if t < TT - 1:
                            emit_hB(3)
                        if layer == 2:
                            emit_c3(3)
                            c3_prev = (t, c3A, c3B)

                        if DBG:
                            nc.sync.dma_start(d_dbg[layer][:][t, :, :],
                                              stg[64:128, 0:FRAME])

                # last frame's conv3d, then sum shifted planes + sigmoid
                conv3d_block(*c3_prev)
                for chunk in range(NCHUNK):
                    pYt = psp.tile([128, CHUNK], F32, tag="psA")
                    pY = pYt[0:TT, :]
                    nc.tensor.matmul(
                        pY, bdt[:],
                        ring[:, chunk * CHUNK:(chunk + 1) * CHUNK],
                        start=True, stop=True)
                    ystg = c3s.tile([TT, CHUNK], F32, tag="ystg")
                    nc.scalar.activation(ystg[:], pY, AF.Sigmoid,
                                         bias=b3t[:])
                    nc.sync.dma_start(
                        d_y[:][:, chunk * CHUNK:(chunk + 1) * CHUNK],
                        ystg[:])

    nc.compile()
    return nc


def prep_inputs(x, k0, rk0, b0, g0, be0, k1, rk1, b1, g1, be1,
                k2, rk2, b2, g2, be2, w3, b3, TT=T):
    x = np.asarray(x, np.float32)
    wti, wtr, _ = pack_weights(
        [np.asarray(k0, np.float32), np.asarray(k1, np.float32),
         np.asarray(k2, np.float32)],
        [np.asarray(rk0, np.float32), np.asarray(rk1, np.float32),
         np.asarray(rk2, np.float32)])
    w3t, _ = pack_w3(np.asarray(w3, np.float32))
    b_all = np.stack([np.asarray(b0, np.float32),
                      np.asarray(b1, np.float32),
                      np.asarray(b2, np.float32)])
    scale = np.float32(1.0 / np.sqrt(1.0 + 1e-3))
    gb_all = np.stack([np.asarray(g0, np.float32) * scale,
                       np.asarray(g1, np.float32) * scale,
                       np.asarray(g2, np.float32) * scale])
    be_all = np.stack([np.asarray(be0, np.float32),
                       np.asarray(be1, np.float32),
                       np.asarray(be2, np.float32)])
    bd = np.zeros((3 * TT, TT), np.float32)
    for m in range(3):
        for t in range(TT):
            bd[m * TT + t, t] = 1.0
    b3b = np.full((TT, 1), np.asarray(b3, np.float32).ravel()[0], np.float32)

    shared = dict(wti=wti, wtr=wtr, w3t=w3t, bd=bd.astype(BF16),
                  b_all=b_all, gb_all=gb_all, be_all=be_all, b3b=b3b)
    in_maps = []
    for bb in range(B):
        xi = np.zeros((1, XLEN), BF16)
        fr = np.zeros((TT, PW, PW), np.float32)
        fr[:, 1:H + 1, 1:W + 1] = x[bb, :TT, :, :, 0]
        xi[0, XLEAD:XLEAD + TT * FRAME] = fr.reshape(-1).astype(BF16)
        m = dict(shared)
        m["x_im"] = xi
        in_maps.append(m)
    return in_maps


_CACHED = {}


def kernel(**inputs):
    from concourse.bass_utils import run_bass_kernel_spmd
    if 'nc' not in _CACHED:
        _CACHED['nc'] = build_nc(T)
    nc = _CACHED['nc']
    in_maps = prep_inputs(**inputs)
    res = run_bass_kernel_spmd(nc, in_maps, core_ids=list(range(B)),
                               trace=bool(os.environ.get('KTRACE')))
    _CACHED['last_res'] = res
    y = np.stack([r["y"].reshape(T, H, W, 1) for r in res.results])
    return y
